# revision 10
# baseline (speedup 1.0000x reference)
"""DRM attention kernel for 8 Trainium2 NeuronCores.

Sharding: B*H = 32 head-slices; core c handles batch b = c//4 and the 4
heads [4*(c%4), 4*(c%4)+4). Weights replicated (pre-sliced per core on
host). Each core computes its 4 heads' attention output through Wo,
producing a partial [T, DM] for its batch; host sums the 4 partials per
batch.

Score work in transposed layout S^T[j, i] (j = key pos on partitions,
i = query pos on free dim).

Math (per head):
  dist(i,j) = |qm_i-km_j|^2 + sum_r (U_i^T(qm_i-km_j))_r^2
Softmax over j is invariant to any additive term that depends only on
i, so the |qm_i|^2 + |Uq_i|^2 part of the expansion is DROPPED.  What
remains:
  S[j,i] = -2 km_j.(qm_i + w'_i)          (K=32 matmul, w' = U_i Uq_i)
         + |km_j|^2                       (per-j: folded into exp bias)
         + sum_r Uk_ijr^2                 (4 K=32 matmuls, squared)
  p = exp(-(S)/t);  denominator via ones column in V.

Tensor-engine discipline: all K=32 matmuls run in the (32,128) tiling
mode with explicit tile_position so 4 of them occupy the four 32-row
strips of the PE concurrently (uu/kmrep spans, uk spans), and mode
switches (which drain the PE) only happen at a handful of phase
boundaries.
"""

import numpy as np

B, T, DM = 2, 512, 1024
H, DH = 16, 64
D, R = 32, 4
TEMP_MIN = 0.5
NCORE = 8
HPC = 4          # heads per core
TC = 4           # 128-chunks along T (key blocks)

# score units per head: (jc, i0, ni), one per key block jc
UNITS = [(_jc, 128 * _jc, T - 128 * _jc) for _jc in range(TC)]
NU = len(UNITS)   # 4

_CACHE = {}


def _build():
    import concourse.bass as bass
    import concourse.tile as tile
    from concourse import mybir, bacc

    f32 = mybir.dt.float32
    PSUM = bass.MemorySpace.PSUM
    Alu = mybir.AluOpType
    Act = mybir.ActivationFunctionType

    nc = bacc.Bacc("TRN2", target_bir_lowering=False, debug=False)
    f32r = mybir.dt.float32r

    def mm(out, lhsT, rhs, **kw):
        nc.tensor.matmul(out, lhsT.bitcast(f32r), rhs.bitcast(f32r), **kw)

    xt_d = nc.dram_tensor("xt", [DM, T], f32r, kind="ExternalInput")
    wqk_d = nc.dram_tensor("wqk", [DM, 512], f32r, kind="ExternalInput")
    wv_d = nc.dram_tensor("wv", [DM, 256], f32r, kind="ExternalInput")
    wo_d = nc.dram_tensor("wo", [256, DM], f32r, kind="ExternalInput")
    bqkA_d = nc.dram_tensor("bqkA", [128, 128], f32r, kind="ExternalInput")
    bqkB_d = nc.dram_tensor("bqkB", [128, 128], f32r, kind="ExternalInput")
    wm4_d = nc.dram_tensor("wm4", [128, 128], f32r, kind="ExternalInput")
    i4rep4_d = nc.dram_tensor("i4rep4", [128, 128], f32r, kind="ExternalInput")
    cosr_d = nc.dram_tensor("cosr", [128, T], f32, kind="ExternalInput")
    sinr_d = nc.dram_tensor("sinr", [128, T], f32, kind="ExternalInput")
    maskd_d = nc.dram_tensor("maskd", [128, 128], f32, kind="ExternalInput")
    nit_d = nc.dram_tensor("nit", [128, 1], f32, kind="ExternalInput")
    negit_d = nc.dram_tensor("negit", [128, 2], f32r, kind="ExternalInput")
    gsum_d = nc.dram_tensor("gsum", [128, 128], f32r, kind="ExternalInput")
    bsum_d = nc.dram_tensor("bsum", [128, D], f32r, kind="ExternalInput")
    y_d = nc.dram_tensor("y", [T, DM], f32, kind="ExternalOutput")

    with tile.TileContext(nc) as tc:
        with (
            tc.tile_pool(name="const", bufs=1) as cpool,
            tc.tile_pool(name="rope", bufs=4) as rpool,
            tc.tile_pool(name="qkm", bufs=2) as qkmpool,
            tc.tile_pool(name="uu", bufs=4) as uupool,
            tc.tile_pool(name="kmr", bufs=4) as kmrpool,
            tc.tile_pool(name="hb", bufs=1) as hbpool,     # per-head [128,T] persistents
            tc.tile_pool(name="scr", bufs=6) as scr,
            tc.tile_pool(name="sq", bufs=4) as sqpool,
            tc.tile_pool(name="pt", bufs=10) as ptpool,
            tc.tile_pool(name="ob", bufs=1) as obpool,
            tc.tile_pool(name="psUK", bufs=1, space=PSUM) as psUK,
            tc.tile_pool(name="psC", bufs=2, space=PSUM) as psC,
            tc.tile_pool(name="psD", bufs=2, space=PSUM) as psD,
        ):
            # ---- constants / weights ----
            xt = [cpool.tile([128, T], f32r, tag=f"xt{k}", name=f"xt{k}") for k in range(8)]
            wqk = [cpool.tile([128, 512], f32r, tag=f"wqk{k}", name=f"wqk{k}") for k in range(8)]
            wv = [cpool.tile([128, 256], f32r, tag=f"wv{k}", name=f"wv{k}") for k in range(8)]
            wo = [cpool.tile([128, DM], f32r, tag=f"wo{p}", name=f"wo{p}") for p in range(2)]
            bqkA = cpool.tile([128, 128], f32r, tag="bqkA")
            bqkB = cpool.tile([128, 128], f32r, tag="bqkB")
            wm4 = cpool.tile([128, 128], f32r, tag="wm4")
            i4rep4 = cpool.tile([128, 128], f32r, tag="i4rep4")
            cosr = cpool.tile([128, T], f32, tag="cosr")
            sinr = cpool.tile([128, T], f32, tag="sinr")
            maskd = cpool.tile([128, 128], f32, tag="maskd")
            nit = cpool.tile([128, 1], f32, tag="nit")
            negit = cpool.tile([128, 2], f32r, tag="negit")
            gsum = cpool.tile([128, 128], f32r, tag="gsum")
            bsum = cpool.tile([128, D], f32r, tag="bsum")
            ones128 = cpool.tile([1, 128], f32r, tag="ones128")
            warm = cpool.tile([128, 512], f32r, tag="warm")
            vext = cpool.tile([128, TC, 260], f32r, tag="vext")

            xt_r = xt_d.ap().rearrange("(k p) t -> k p t", p=128)
            wqk_r = wqk_d.ap().rearrange("(k p) m -> k p m", p=128)
            wv_r = wv_d.ap().rearrange("(k p) m -> k p m", p=128)
            wo_r = wo_d.ap().rearrange("(k p) m -> k p m", p=128)
            for k in range(8):
                nc.sync.dma_start(xt[k][:], xt_r[k])
                nc.sync.dma_start(wqk[k][:], wqk_r[k])
            nc.sync.dma_start(cosr[:], cosr_d.ap())
            nc.sync.dma_start(sinr[:], sinr_d.ap())
            nc.sync.dma_start(bqkA[:], bqkA_d.ap())
            nc.sync.dma_start(bqkB[:], bqkB_d.ap())
            nc.sync.dma_start(wm4[:], wm4_d.ap())
            nc.sync.dma_start(i4rep4[:], i4rep4_d.ap())
            nc.sync.dma_start(gsum[:], gsum_d.ap())
            nc.sync.dma_start(bsum[:], bsum_d.ap())
            nc.sync.dma_start(maskd[:], maskd_d.ap())
            nc.sync.dma_start(nit[:], nit_d.ap())
            nc.sync.dma_start(negit[:], negit_d.ap())
            for k in range(8):
                nc.sync.dma_start(wv[k][:], wv_r[k])
            for p in range(2):
                nc.sync.dma_start(wo[p][:], wo_r[p])
            nc.gpsimd.memset(ones128[:].bitcast(f32), 1.0)
            nc.gpsimd.memset(warm[:].bitcast(f32), 0.0)
            nc.gpsimd.memset(vext[:].bitcast(f32), 1.0)

            # PE warm-up: ~8us of dummy matmuls while input DMAs stream, so
            # the HAM clock-gate reaches K=8/8 before real work arrives.
            warm_ps = psD.tile([128, 512], f32, tag="psD", name="warm_ps")
            for w in range(20):
                mm(warm_ps[:], warm[:, :128], warm[:],
                   start=(w == 0), stop=(w == 19))

            # ================= phase 1: (128,128) projections =================
            # QK projection + RoPE + qm/km sigmoid, per pair
            qkm_sig = []     # per pair: [128,T] = [qm_e0; qm_e1; km_e0; km_e1]
            for p in range(2):
                ropeAB = []
                for s in range(2):      # dh half: A (first 32) / B (second)
                    m = 2 * p + s
                    qk_ps = psC.tile([128, T], f32, tag="psC", name="qk_ps")
                    for k in range(8):
                        mm(qk_ps[:], wqk[k][:, m * 128:(m + 1) * 128], xt[k][:],
                           start=(k == 0), stop=(k == 7))
                    ropeAB.append(qk_ps)
                A, Bt = ropeAB
                m1 = scr.tile([128, T], f32, tag="scr", bufs=4)
                m2 = scr.tile([128, T], f32, tag="scr", bufs=4)
                nc.vector.tensor_mul(m1[:], A[:], cosr[:])
                nc.vector.tensor_mul(m2[:], Bt[:], sinr[:])
                ropeA = rpool.tile([128, T], f32r, tag="rope")
                nc.vector.tensor_sub(ropeA[:], m1[:], m2[:])
                m3 = scr.tile([128, T], f32, tag="scr", bufs=4)
                m4 = scr.tile([128, T], f32, tag="scr", bufs=4)
                nc.vector.tensor_mul(m3[:], A[:], sinr[:])
                nc.vector.tensor_mul(m4[:], Bt[:], cosr[:])
                ropeB = rpool.tile([128, T], f32r, tag="rope")
                nc.vector.tensor_add(ropeB[:], m3[:], m4[:])

                qkm_ps = psD.tile([128, T], f32, tag="psD", name="qkm_ps")
                mm(qkm_ps[:], bqkA[:], ropeA[:], start=True, stop=False)
                mm(qkm_ps[:], bqkB[:], ropeB[:], start=False, stop=True)
                sig = qkmpool.tile([128, T], f32r, tag="qkm")
                nc.scalar.activation(sig[:], qkm_ps[:], Act.Sigmoid)
                qkm_sig.append(sig)

            # V projection into [v_h | 1] blocks of vext
            for jc in range(TC):
                v_ps = psC.tile([128, 256], f32, tag="psC", name="v_ps")
                for k in range(8):
                    mm(v_ps[:], xt[k][:, jc * 128:(jc + 1) * 128], wv[k][:],
                       start=(k == 0), stop=(k == 7))
                for hl in range(HPC):
                    nc.vector.tensor_copy(vext[:, jc, hl * 65:hl * 65 + 64],
                                          v_ps[:, hl * 64:(hl + 1) * 64])

            # ============ phase 2: (32,128) spans: uu/kmrep/qmrep ============
            # per pair: span A = {uu_e0@s0, uu_e1@s1, kmrep_e0@s2, kmrep_e1@s3}
            #           span B = {qmrep_e0@s0, qmrep_e1@s1}
            # wm4 rows 0:64 = wm at strips 0,1 ; rows 64:128 = i4rep at 2,3
            uu_sb = [None] * HPC      # [128, T] f32r, rows (32r+d') = U_r
            kmr_sb = [None] * HPC     # [128, T] f32r, km replicated x4
            tmpc_l = [None] * HPC
            for p in range(2):
                sig = qkm_sig[p]
                # quad: seg 0/1 = uu_e0/uu_e1, seg 2/3 = kmr_e0/kmr_e1
                quad = psUK.tile([128, 4, T], f32, tag="quad", name=f"quad{p}")
                for e in range(2):
                    mm(quad[:, e, :], wm4[32 * e:32 * e + 32, :],
                       sig[32 * e:32 * e + 32, :],
                       start=True, stop=True, tile_position=(32 * e, 0),
                       skip_group_check=True)
                for e in range(2):
                    mm(quad[:, 2 + e, :], i4rep4[64 + 32 * e:96 + 32 * e, :],
                       sig[64 + 32 * e:96 + 32 * e, :],
                       start=True, stop=True, tile_position=(64 + 32 * e, 0),
                       skip_group_check=True)
                for e in range(2):
                    h = 2 * p + e
                    u = uupool.tile([128, T], f32r, tag="uu", name=f"uu{h}")
                    nc.vector.tensor_copy(u[:], quad[:, e, :])
                    uu_sb[h] = u
                    kr = kmrpool.tile([128, T], f32r, tag="kmr", name=f"kmr{h}")
                    nc.vector.tensor_copy(kr[:], quad[:, 2 + e, :])
                    kmr_sb[h] = kr
                for e in range(2):
                    h = 2 * p + e
                    pool = psC if e == 0 else psD
                    qm_ps = pool.tile([128, T], f32, tag=pool.name, name=f"qm_ps{h}")
                    mm(qm_ps[:], i4rep4[32 * e:32 * e + 32, :],
                       sig[32 * e:32 * e + 32, :],
                       start=True, stop=True, tile_position=(32 * e, 0))
                    t = scr.tile([128, T], f32r, tag="tmpc", bufs=4)
                    nc.vector.scalar_tensor_tensor(
                        t[:], qm_ps[:], 1.0, uu_sb[h][:],
                        op0=Alu.bypass, op1=Alu.mult)
                    tmpc_l[h] = t

            # ============ phase 3: (128,128) gsum -> uqrep ============
            tmp2_l = []
            for h in range(HPC):
                pool = psC if h % 2 == 0 else psD
                uq_ps = pool.tile([128, T], f32, tag=pool.name, name=f"uq_ps{h}")
                mm(uq_ps[:], gsum[:], tmpc_l[h][:], start=True, stop=True)
                t2 = scr.tile([128, T], f32r, tag="tmp2", bufs=4)
                nc.vector.scalar_tensor_tensor(
                    t2[:], uq_ps[:], 1.0, uu_sb[h][:],
                    op0=Alu.bypass, op1=Alu.mult)
                tmp2_l.append(t2)

            # ============ phase 4: (128,32) bsum -> w' ============
            gt4 = hbpool.tile([128, T], f32r, tag="gt4")
            kme4 = hbpool.tile([128, T], f32r, tag="kme4")
            kmsq4 = hbpool.tile([128, T], f32r, tag="kmsq4")
            for h in range(HPC):
                p, e = h // 2, h % 2
                sig = qkm_sig[p]
                wpt_ps = psC.tile([32, T], f32, tag="psC", name=f"wpt_ps{h}")
                mm(wpt_ps[:], bsum[:], tmp2_l[h][:], start=True, stop=True)
                nc.vector.scalar_tensor_tensor(
                    gt4[32 * h:32 * h + 32, :], wpt_ps[:],
                    1.0, sig[32 * e:32 * e + 32, :],
                    op0=Alu.bypass, op1=Alu.add)
                nc.vector.tensor_scalar_mul(
                    kme4[32 * h:32 * h + 32, :],
                    sig[64 + 32 * e:96 + 32 * e, :], -2.0)
                nc.gpsimd.tensor_mul(
                    kmsq4[32 * h:32 * h + 32, :],
                    sig[64 + 32 * e:96 + 32 * e, :],
                    sig[64 + 32 * e:96 + 32 * e, :])

            # ============ phase 5: (32,128) k2col: -|km|^2/t ============
            # one [128, 16] psum tile; col 4h+jc = bias for (head h, block jc)
            k2q = psUK.tile([128, 4, T], f32, tag="quad", name="k2q")
            for h in range(HPC):
                for jc in range(TC):
                    mm(k2q[:, h, 2 * jc:2 * jc + 2],
                       kmsq4[32 * h:32 * h + 32, jc * 128:(jc + 1) * 128],
                       negit[32 * h:32 * h + 32, :],
                       start=(jc == 0), stop=True, tile_position=(32 * h, 0),
                       skip_group_check=True)
            k2sb = hbpool.tile([128, 4, 8], f32, tag="k2sb")
            nc.vector.tensor_copy(k2sb[:], k2q[:, :, 0:8])

            # ============ phase 6: score loop ============
            # per (head, unit): (32,128)-mode spans {uk x4 strips, lin@strip h}
            # then elementwise chain, then (128,128) attnV interleaved.
            ot_ps_l = [None] * HPC
            pt_tiles = [[None] * NU for _ in range(HPC)]

            def score_unit(h, u):
                jc, i0, ni = UNITS[u]
                uk = psUK.tile([128, 4, T], f32, tag="quad", name="uk")
                s_ps = psC.tile([128, 512], f32, tag="psC", name="s_ps")
                for r in range(R):
                    mm(uk[:, r, :ni],
                       kmr_sb[h][32 * r:32 * r + 32, jc * 128:(jc + 1) * 128],
                       uu_sb[h][32 * r:32 * r + 32, i0:i0 + ni],
                       start=True, stop=True,
                       tile_position=(32 * r, 0),
                       skip_group_check=True)
                mm(s_ps[:, :ni],
                   kme4[32 * h:32 * h + 32, jc * 128:(jc + 1) * 128],
                   gt4[32 * h:32 * h + 32, i0:i0 + ni],
                   start=True, stop=True, tile_position=(32 * h, 0))
                # elementwise: one big square (ACT) ; adds (gpsimd/DVE)
                sq4 = sqpool.tile([128, 4, 512], f32, tag="sq4", bufs=2)
                nc.scalar.square(sq4[:, :, :ni], uk[:, :, :ni])
                dd = scr.tile([128, 2, 512], f32, tag="dd", bufs=2)
                nc.gpsimd.tensor_add(dd[:, :, :ni], sq4[:, 0:2, :ni],
                                     sq4[:, 2:4, :ni])
                cc = scr.tile([128, 512], f32, tag="cc", bufs=2)
                nc.vector.tensor_add(cc[:, :ni], dd[:, 0, :ni], dd[:, 1, :ni])
                ssb = scr.tile([128, 512], f32, tag="ssb", bufs=2)
                nc.vector.scalar_tensor_tensor(
                    ssb[:, :ni], s_ps[:, :ni], 1.0, cc[:, :ni],
                    op0=Alu.bypass, op1=Alu.add)
                pt = ptpool.tile([128, 512], f32r, tag="pt")
                nc.scalar.activation(pt[:, :ni], ssb[:, :ni], Act.Exp,
                                     bias=k2sb[:, h, 2 * jc:2 * jc + 1],
                                     scale=nit[:, 0:1])
                nc.vector.tensor_mul(pt[:, :128], pt[:, :128], maskd[:])
                pt_tiles[h][u] = pt

            def attn_unit(h, u):
                jc, i0, ni = UNITS[u]
                if u == 0:
                    ot_ps_l[h] = psD.tile([128, T], f32, tag="psD",
                                          name=f"ot{h}")
                mm(ot_ps_l[h][:65, i0:i0 + ni],
                   vext[:, jc, h * 65:(h + 1) * 65],
                   pt_tiles[h][u][:, :ni],
                   start=(jc == 0), stop=(u == NU - 1),
                   skip_group_check=True)
                pt_tiles[h][u] = None

            num_sb = [None] * HPC
            den_sb = [None] * HPC
            bc_ps_l = [None] * HPC
            stacked = []
            for p in range(2):
                stk = hbpool.tile([128, T], f32r, tag=f"stk{p}", name=f"stk{p}")
                stacked.append(stk)

            def drain_head(h):
                ns = obpool.tile([64, T], f32r, tag=f"num{h}", name=f"num{h}")
                nc.vector.tensor_copy(ns[:], ot_ps_l[h][:64, :])
                ds = obpool.tile([1, T], f32r, tag=f"den{h}", name=f"den{h}")
                nc.scalar.copy(ds[:], ot_ps_l[h][64:65, :])
                num_sb[h], den_sb[h] = ns, ds

            def bc_head(h):
                # K=1 broadcast in (32,128) mode so it rides the score stream
                bc_ps = psC.tile([128, T], f32, tag="psC", name=f"bc{h}")
                mm(bc_ps[:], ones128[:], den_sb[h][:], start=True, stop=True)
                bc_ps_l[h] = bc_ps

            def norm_head(h):
                p, e = h // 2, h % 2
                rec = scr.tile([64, T], f32, tag="rec", bufs=2)
                nc.vector.reciprocal_approx_fast(out=rec[:],
                                                 in_=bc_ps_l[h][0:64, :])
                nc.vector.tensor_mul(stacked[p][64 * e:64 * e + 64, :],
                                     num_sb[h][:], rec[:])

            # interleave: score units of head h with attnV units of head h-1
            for u in range(NU):
                score_unit(0, u)
            for u in range(NU):
                score_unit(1, u)
                attn_unit(0, u)
            drain_head(0)
            for u in range(NU):
                score_unit(2, u)
                attn_unit(1, u)
                if u == 0:
                    bc_head(0)
            drain_head(1)
            norm_head(0)
            for u in range(NU):
                score_unit(3, u)
                attn_unit(2, u)
                if u == 0:
                    bc_head(1)
            drain_head(2)
            norm_head(1)
            for u in range(NU):
                attn_unit(3, u)
                if u == 0:
                    bc_head(2)
            drain_head(3)
            norm_head(2)
            bc_head(3)
            norm_head(3)

            # ============ phase 8: (128,128) output projection ============
            for ic in range(TC):
                for ncn in range(2):
                    y_ps = psC.tile([128, 512], f32, tag="psC", name="y_ps")
                    mm(y_ps[:], stacked[0][:, ic * 128:(ic + 1) * 128],
                       wo[0][:, ncn * 512:(ncn + 1) * 512],
                       start=True, stop=False)
                    mm(y_ps[:], stacked[1][:, ic * 128:(ic + 1) * 128],
                       wo[1][:, ncn * 512:(ncn + 1) * 512],
                       start=False, stop=True)
                    y_sb = scr.tile([128, 512], f32, tag="ysb", bufs=2)
                    nc.scalar.copy(y_sb[:], y_ps[:])
                    nc.sync.dma_start(
                        y_d.ap()[ic * 128:(ic + 1) * 128,
                                 ncn * 512:(ncn + 1) * 512],
                        y_sb[:])

    nc.compile()
    return nc


def _r32(a):
    """Round fp32 to fp32r (11-bit mantissa, RNE)."""
    u = np.ascontiguousarray(a, np.float32).view(np.uint32).astype(np.uint64)
    u = (u + 0x7FF + ((u >> 12) & 1)) & 0xFFFFF000
    return u.astype(np.uint32).view(np.float32)


def _rope_tables():
    inv_freq = 1.0 / (10000.0 ** (np.arange(0, DH, 2, dtype=np.float32) / DH))
    t = np.arange(T, dtype=np.float32)
    freqs = t[:, None] * inv_freq[None, :]          # [T, 32]
    return np.cos(freqs), np.sin(freqs)


def _prep_inputs(x, Wq, Wk, Wv, Wo, Wqm, Wkm, Wmetric, temperature):
    x = np.asarray(x, np.float32)
    Wq, Wk, Wv, Wo = (np.asarray(w, np.float32) for w in (Wq, Wk, Wv, Wo))
    Wqm, Wkm = np.asarray(Wqm, np.float32), np.asarray(Wkm, np.float32)
    Wmetric = np.asarray(Wmetric, np.float32)
    temp = float(np.asarray(temperature))

    cosf, sinf = _rope_tables()
    cosr = np.ascontiguousarray(np.tile(cosf.T, (4, 1)))   # [128, T]
    sinr = np.ascontiguousarray(np.tile(sinf.T, (4, 1)))

    bqkA = np.zeros((128, 128), np.float32)
    bqkB = np.zeros((128, 128), np.float32)
    for ee in range(2):
        bqkA[64 * ee:64 * ee + 32, 32 * ee:32 * ee + 32] = Wqm[0:32]
        bqkA[64 * ee + 32:64 * ee + 64, 64 + 32 * ee:96 + 32 * ee] = Wkm[0:32]
        bqkB[64 * ee:64 * ee + 32, 32 * ee:32 * ee + 32] = Wqm[32:64]
        bqkB[64 * ee + 32:64 * ee + 64, 64 + 32 * ee:96 + 32 * ee] = Wkm[32:64]

    wm = np.ascontiguousarray(
        Wmetric.reshape(D, D, R).transpose(0, 2, 1).reshape(D, D * R))
    i4rep = np.ascontiguousarray(np.tile(np.eye(D, dtype=np.float32), (1, 4)))
    wm4 = np.zeros((128, 128), np.float32)
    wm4[0:32] = wm
    wm4[32:64] = wm
    wm4[64:96] = i4rep
    wm4[96:128] = i4rep
    i4rep4 = np.zeros((128, 128), np.float32)
    for s in range(4):
        i4rep4[32 * s:32 * s + 32] = i4rep

    jj, ii = np.meshgrid(np.arange(128), np.arange(128), indexing="ij")
    maskd = (jj <= ii).astype(np.float32)
    it = -1.0 / max(temp, TEMP_MIN)
    nit = np.full((128, 1), it, np.float32)
    negit = np.full((128, 2), it, np.float32)
    gsum = np.zeros((128, 128), np.float32)   # [(r',d'), (r,d)] = [r'==r]
    for a in range(128):
        for bcol in range(128):
            if a // 32 == bcol // 32:
                gsum[a, bcol] = 1.0
    bsum = np.zeros((128, D), np.float32)     # [(r,d), d'] = [d==d']
    for a in range(128):
        bsum[a, a % 32] = 1.0

    in_maps = []
    for c in range(NCORE):
        b, g = c // 4, c % 4
        lh0 = 4 * g
        wqk = np.empty((DM, 512), np.float32)
        for p in range(2):
            for s in range(2):
                m = 2 * p + s
                for ee in range(2):
                    h = lh0 + 2 * p + ee
                    cq = Wq[:, h * 64 + 32 * s: h * 64 + 32 * s + 32]
                    ck = Wk[:, h * 64 + 32 * s: h * 64 + 32 * s + 32]
                    wqk[:, m * 128 + 64 * ee: m * 128 + 64 * ee + 32] = cq
                    wqk[:, m * 128 + 64 * ee + 32: m * 128 + 64 * ee + 64] = ck
        in_maps.append({
            "xt": _r32(x[b].T),
            "wqk": _r32(wqk),
            "wv": _r32(Wv[:, lh0 * 64: lh0 * 64 + 256]),
            "wo": _r32(Wo[lh0 * 64: lh0 * 64 + 256, :]),
            "bqkA": _r32(bqkA),
            "bqkB": _r32(bqkB),
            "wm4": _r32(wm4),
            "i4rep4": i4rep4,
            "cosr": cosr,
            "sinr": sinr,
            "maskd": maskd,
            "nit": nit,
            "negit": negit,
            "gsum": gsum,
            "bsum": bsum,
        })
    return in_maps


def kernel(x, Wq, Wk, Wv, Wo, Wqm, Wkm, Wmetric, temperature, **_):
    from concourse import bass_utils

    if "nc" not in _CACHE:
        _CACHE["nc"] = _build()
    nc = _CACHE["nc"]

    in_maps = _prep_inputs(x, Wq, Wk, Wv, Wo, Wqm, Wkm, Wmetric, temperature)
    res = bass_utils.run_bass_kernel_spmd(nc, in_maps,
                                          core_ids=list(range(NCORE)))
    y = np.zeros((B, T, DM), np.float32)
    for b in range(B):
        acc = res.results[4 * b]["y"].astype(np.float32)
        for g in range(1, 4):
            acc = acc + res.results[4 * b + g]["y"]
        y[b] = acc
    return y


# revision 12
# speedup vs baseline: 1.2258x; 1.2258x over previous
"""DRM attention kernel for 8 Trainium2 NeuronCores.

Sharding: B*H = 32 head-slices; core c handles batch b = c//4 and the 4
heads [4*(c%4), 4*(c%4)+4). Weights replicated (pre-sliced per core on
host). Each core computes its 4 heads' attention output through Wo,
producing a partial [T, DM] for its batch; host sums the 4 partials per
batch.

Score work in transposed layout S^T[j, i] (j = key pos on partitions,
i = query pos on free dim).

Math (per head):
  dist(i,j) = |qm_i-km_j|^2 + sum_r (U_i^T(qm_i-km_j))_r^2
Softmax over j is invariant to any additive term that depends only on
i, so the |qm_i|^2 + |Uq_i|^2 part of the expansion is DROPPED.  What
remains:
  S[j,i] = -2 km_j.(qm_i + w'_i)          (K=32 matmul, w' = U_i Uq_i)
         + |km_j|^2                       (per-j: folded into exp bias)
         + sum_r Uk_ijr^2                 (4 K=32 matmuls, squared)
  p = exp(-(S)/t);  denominator via ones column in V.

Tensor-engine discipline: all K=32 matmuls run in the (32,128) tiling
mode with explicit tile_position so 4 of them occupy the four 32-row
strips of the PE concurrently (uu/kmrep spans, uk spans), and mode
switches (which drain the PE) only happen at a handful of phase
boundaries.
"""

import numpy as np

B, T, DM = 2, 512, 1024
H, DH = 16, 64
D, R = 32, 4
TEMP_MIN = 0.5
NCORE = 8
HPC = 4          # heads per core
TC = 4           # 128-chunks along T (key blocks)

# score units per head: (jc, i0, ni), one per key block jc
UNITS = [(_jc, 128 * _jc, T - 128 * _jc) for _jc in range(TC)]
NU = len(UNITS)   # 4

_CACHE = {}


def _build():
    import concourse.bass as bass
    import concourse.tile as tile
    from concourse import mybir, bacc

    f32 = mybir.dt.float32
    PSUM = bass.MemorySpace.PSUM
    Alu = mybir.AluOpType
    Act = mybir.ActivationFunctionType

    nc = bacc.Bacc("TRN2", target_bir_lowering=False, debug=False)
    f32r = mybir.dt.float32r

    def mm(out, lhsT, rhs, **kw):
        nc.tensor.matmul(out, lhsT.bitcast(f32r), rhs.bitcast(f32r), **kw)

    xt_d = nc.dram_tensor("xt", [DM, T], f32r, kind="ExternalInput")
    wqk_d = nc.dram_tensor("wqk", [DM, 512], f32r, kind="ExternalInput")
    wv_d = nc.dram_tensor("wv", [DM, 256], f32r, kind="ExternalInput")
    wo_d = nc.dram_tensor("wo", [256, DM], f32r, kind="ExternalInput")
    bqkA_d = nc.dram_tensor("bqkA", [128, 128], f32r, kind="ExternalInput")
    bqkB_d = nc.dram_tensor("bqkB", [128, 128], f32r, kind="ExternalInput")
    wm4_d = nc.dram_tensor("wm4", [128, 128], f32r, kind="ExternalInput")
    i4rep4_d = nc.dram_tensor("i4rep4", [128, 128], f32r, kind="ExternalInput")
    cosr_d = nc.dram_tensor("cosr", [128, T], f32, kind="ExternalInput")
    sinr_d = nc.dram_tensor("sinr", [128, T], f32, kind="ExternalInput")
    maskd_d = nc.dram_tensor("maskd", [128, 128], f32, kind="ExternalInput")
    nit_d = nc.dram_tensor("nit", [128, 1], f32, kind="ExternalInput")
    negit_d = nc.dram_tensor("negit", [128, 2], f32r, kind="ExternalInput")
    gsum_d = nc.dram_tensor("gsum", [128, 128], f32r, kind="ExternalInput")
    bsum_d = nc.dram_tensor("bsum", [128, D], f32r, kind="ExternalInput")
    y_d = nc.dram_tensor("y", [T, DM], f32, kind="ExternalOutput")

    with tile.TileContext(nc) as tc:
        with (
            tc.tile_pool(name="const", bufs=1) as cpool,
            tc.tile_pool(name="rope", bufs=4) as rpool,
            tc.tile_pool(name="qkm", bufs=2) as qkmpool,
            tc.tile_pool(name="uu", bufs=4) as uupool,
            tc.tile_pool(name="kmr", bufs=4) as kmrpool,
            tc.tile_pool(name="hb", bufs=1) as hbpool,     # per-head [128,T] persistents
            tc.tile_pool(name="scr", bufs=6) as scr,
            tc.tile_pool(name="sq", bufs=4) as sqpool,
            tc.tile_pool(name="pt", bufs=10) as ptpool,
            tc.tile_pool(name="ob", bufs=1) as obpool,
            tc.tile_pool(name="psUK", bufs=1, space=PSUM) as psUK,
            tc.tile_pool(name="psC", bufs=2, space=PSUM) as psC,
            tc.tile_pool(name="psD", bufs=2, space=PSUM) as psD,
        ):
            # ---- constants / weights ----
            xt = [cpool.tile([128, T], f32r, tag=f"xt{k}", name=f"xt{k}") for k in range(8)]
            wqk = [cpool.tile([128, 512], f32r, tag=f"wqk{k}", name=f"wqk{k}") for k in range(8)]
            wv = [cpool.tile([128, 256], f32r, tag=f"wv{k}", name=f"wv{k}") for k in range(8)]
            wo = [cpool.tile([128, DM], f32r, tag=f"wo{p}", name=f"wo{p}") for p in range(2)]
            bqkA = cpool.tile([128, 128], f32r, tag="bqkA")
            bqkB = cpool.tile([128, 128], f32r, tag="bqkB")
            wm4 = cpool.tile([128, 128], f32r, tag="wm4")
            i4rep4 = cpool.tile([128, 128], f32r, tag="i4rep4")
            cosr = cpool.tile([128, T], f32, tag="cosr")
            sinr = cpool.tile([128, T], f32, tag="sinr")
            maskd = cpool.tile([128, 128], f32, tag="maskd")
            nit = cpool.tile([128, 1], f32, tag="nit")
            negit = cpool.tile([128, 2], f32r, tag="negit")
            gsum = cpool.tile([128, 128], f32r, tag="gsum")
            bsum = cpool.tile([128, D], f32r, tag="bsum")
            ones128 = cpool.tile([1, 128], f32r, tag="ones128")
            warm = cpool.tile([128, 512], f32r, tag="warm")
            vext = cpool.tile([128, TC, 260], f32r, tag="vext")

            xt_r = xt_d.ap().rearrange("(k p) t -> k p t", p=128)
            wqk_r = wqk_d.ap().rearrange("(k p) m -> k p m", p=128)
            wv_r = wv_d.ap().rearrange("(k p) m -> k p m", p=128)
            wo_r = wo_d.ap().rearrange("(k p) m -> k p m", p=128)
            for k in range(8):
                nc.sync.dma_start(xt[k][:], xt_r[k])
                nc.sync.dma_start(wqk[k][:], wqk_r[k])
            nc.sync.dma_start(cosr[:], cosr_d.ap())
            nc.sync.dma_start(sinr[:], sinr_d.ap())
            nc.sync.dma_start(bqkA[:], bqkA_d.ap())
            nc.sync.dma_start(bqkB[:], bqkB_d.ap())
            nc.sync.dma_start(wm4[:], wm4_d.ap())
            nc.sync.dma_start(i4rep4[:], i4rep4_d.ap())
            nc.sync.dma_start(gsum[:], gsum_d.ap())
            nc.sync.dma_start(bsum[:], bsum_d.ap())
            nc.sync.dma_start(maskd[:], maskd_d.ap())
            nc.sync.dma_start(nit[:], nit_d.ap())
            nc.sync.dma_start(negit[:], negit_d.ap())
            for k in range(8):
                nc.sync.dma_start(wv[k][:], wv_r[k])
            for p in range(2):
                nc.sync.dma_start(wo[p][:], wo_r[p])
            nc.gpsimd.memset(ones128[:].bitcast(f32), 1.0)
            nc.gpsimd.memset(warm[:].bitcast(f32), 0.0)
            nc.gpsimd.memset(vext[:].bitcast(f32), 1.0)

            # PE warm-up: ~8us of dummy matmuls while input DMAs stream, so
            # the HAM clock-gate reaches K=8/8 before real work arrives.
            warm_ps = psD.tile([128, 512], f32, tag="psD", name="warm_ps")
            for w in range(20):
                mm(warm_ps[:], warm[:, :128], warm[:],
                   start=(w == 0), stop=(w == 19))

            # ================= phase 1: (128,128) projections =================
            # QK projection + RoPE + qm/km sigmoid, per pair
            qkm_sig = []     # per pair: [128,T] = [qm_e0; qm_e1; km_e0; km_e1]
            for p in range(2):
                ropeAB = []
                for s in range(2):      # dh half: A (first 32) / B (second)
                    m = 2 * p + s
                    qk_ps = psC.tile([128, T], f32, tag="psC", name="qk_ps")
                    for k in range(8):
                        mm(qk_ps[:], wqk[k][:, m * 128:(m + 1) * 128], xt[k][:],
                           start=(k == 0), stop=(k == 7))
                    ropeAB.append(qk_ps)
                A, Bt = ropeAB
                m1 = scr.tile([128, T], f32, tag="scr", bufs=4)
                m2 = scr.tile([128, T], f32, tag="scr", bufs=4)
                nc.vector.tensor_mul(m1[:], A[:], cosr[:])
                nc.vector.tensor_mul(m2[:], Bt[:], sinr[:])
                ropeA = rpool.tile([128, T], f32r, tag="rope")
                nc.vector.tensor_sub(ropeA[:], m1[:], m2[:])
                m3 = scr.tile([128, T], f32, tag="scr", bufs=4)
                m4 = scr.tile([128, T], f32, tag="scr", bufs=4)
                nc.vector.tensor_mul(m3[:], A[:], sinr[:])
                nc.vector.tensor_mul(m4[:], Bt[:], cosr[:])
                ropeB = rpool.tile([128, T], f32r, tag="rope")
                nc.vector.tensor_add(ropeB[:], m3[:], m4[:])

                qkm_ps = psD.tile([128, T], f32, tag="psD", name="qkm_ps")
                mm(qkm_ps[:], bqkA[:], ropeA[:], start=True, stop=False)
                mm(qkm_ps[:], bqkB[:], ropeB[:], start=False, stop=True)
                sig = qkmpool.tile([128, T], f32r, tag="qkm")
                nc.scalar.activation(sig[:], qkm_ps[:], Act.Sigmoid)
                qkm_sig.append(sig)

            # V projection into [v_h | 1] blocks of vext
            for jc in range(TC):
                v_ps = psC.tile([128, 256], f32, tag="psC", name="v_ps")
                for k in range(8):
                    mm(v_ps[:], xt[k][:, jc * 128:(jc + 1) * 128], wv[k][:],
                       start=(k == 0), stop=(k == 7))
                for hl in range(HPC):
                    nc.vector.tensor_copy(vext[:, jc, hl * 65:hl * 65 + 64],
                                          v_ps[:, hl * 64:(hl + 1) * 64])

            # ============ phase 2: (32,128) spans: uu/kmrep/qmrep ============
            # per pair: span A = {uu_e0@s0, uu_e1@s1, kmrep_e0@s2, kmrep_e1@s3}
            #           span B = {qmrep_e0@s0, qmrep_e1@s1}
            # wm4 rows 0:64 = wm at strips 0,1 ; rows 64:128 = i4rep at 2,3
            uu_sb = [None] * HPC      # [128, T] f32r, rows (32r+d') = U_r
            kmr_sb = [None] * HPC     # [128, T] f32r, km replicated x4
            tmpc_l = [None] * HPC
            for p in range(2):
                sig = qkm_sig[p]
                # quad: seg 0/1 = uu_e0/uu_e1, seg 2/3 = kmr_e0/kmr_e1
                quad = psUK.tile([128, 4, T], f32, tag="quad", name=f"quad{p}")
                for e in range(2):
                    mm(quad[:, e, :], wm4[32 * e:32 * e + 32, :],
                       sig[32 * e:32 * e + 32, :],
                       start=True, stop=True, tile_position=(32 * e, 0),
                       skip_group_check=True)
                for e in range(2):
                    mm(quad[:, 2 + e, :], i4rep4[64 + 32 * e:96 + 32 * e, :],
                       sig[64 + 32 * e:96 + 32 * e, :],
                       start=True, stop=True, tile_position=(64 + 32 * e, 0),
                       skip_group_check=True)
                for e in range(2):
                    h = 2 * p + e
                    u = uupool.tile([128, T], f32r, tag="uu", name=f"uu{h}")
                    nc.vector.tensor_copy(u[:], quad[:, e, :])
                    uu_sb[h] = u
                    kr = kmrpool.tile([128, T], f32r, tag="kmr", name=f"kmr{h}")
                    nc.vector.tensor_copy(kr[:], quad[:, 2 + e, :])
                    kmr_sb[h] = kr
                for e in range(2):
                    h = 2 * p + e
                    pool = psC if e == 0 else psD
                    qm_ps = pool.tile([128, T], f32, tag=pool.name, name=f"qm_ps{h}")
                    mm(qm_ps[:], i4rep4[32 * e:32 * e + 32, :],
                       sig[32 * e:32 * e + 32, :],
                       start=True, stop=True, tile_position=(32 * e, 0))
                    t = scr.tile([128, T], f32r, tag="tmpc", bufs=4)
                    nc.vector.scalar_tensor_tensor(
                        t[:], qm_ps[:], 1.0, uu_sb[h][:],
                        op0=Alu.bypass, op1=Alu.mult)
                    tmpc_l[h] = t

            # ============ phase 3: (128,128) gsum -> uqrep ============
            tmp2_l = []
            for h in range(HPC):
                pool = psC if h % 2 == 0 else psD
                uq_ps = pool.tile([128, T], f32, tag=pool.name, name=f"uq_ps{h}")
                mm(uq_ps[:], gsum[:], tmpc_l[h][:], start=True, stop=True)
                t2 = scr.tile([128, T], f32r, tag="tmp2", bufs=4)
                nc.vector.scalar_tensor_tensor(
                    t2[:], uq_ps[:], 1.0, uu_sb[h][:],
                    op0=Alu.bypass, op1=Alu.mult)
                tmp2_l.append(t2)

            # ============ phase 4: (128,32) bsum -> w' ============
            gt4 = hbpool.tile([128, T], f32r, tag="gt4")
            kme4 = hbpool.tile([128, T], f32r, tag="kme4")
            kmsq4 = hbpool.tile([128, T], f32r, tag="kmsq4")
            for h in range(HPC):
                p, e = h // 2, h % 2
                sig = qkm_sig[p]
                wpt_ps = psC.tile([32, T], f32, tag="psC", name=f"wpt_ps{h}")
                mm(wpt_ps[:], bsum[:], tmp2_l[h][:], start=True, stop=True)
                nc.vector.scalar_tensor_tensor(
                    gt4[32 * h:32 * h + 32, :], wpt_ps[:],
                    1.0, sig[32 * e:32 * e + 32, :],
                    op0=Alu.bypass, op1=Alu.add)
                nc.vector.tensor_scalar_mul(
                    kme4[32 * h:32 * h + 32, :],
                    sig[64 + 32 * e:96 + 32 * e, :], -2.0)
                nc.gpsimd.tensor_mul(
                    kmsq4[32 * h:32 * h + 32, :],
                    sig[64 + 32 * e:96 + 32 * e, :],
                    sig[64 + 32 * e:96 + 32 * e, :])

            # ============ phase 5: (32,128) k2col: -|km|^2/t ============
            # one [128, 16] psum tile; col 4h+jc = bias for (head h, block jc)
            k2sb = hbpool.tile([128, 4, 8], f32, tag="k2sb")
            for p in range(2):
                k2c = psC.tile([128, 512], f32, tag="psC", name=f"k2c{p}")
                k2d = psD.tile([128, 512], f32, tag="psD", name=f"k2d{p}")
                for e in range(2):
                    h = 2 * p + e
                    dst = k2c if e == 0 else k2d
                    for jc in range(TC):
                        mm(dst[:, 2 * jc:2 * jc + 2],
                           kmsq4[32 * h:32 * h + 32, jc * 128:(jc + 1) * 128],
                           negit[32 * h:32 * h + 32, :],
                           start=(jc == 0), stop=True,
                           tile_position=(32 * h, 0),
                           skip_group_check=True)
                nc.vector.tensor_copy(k2sb[:, 2 * p, :], k2c[:, 0:8])
                nc.vector.tensor_copy(k2sb[:, 2 * p + 1, :], k2d[:, 0:8])

            # ============ phase 6: score loop ============
            # software-pipelined: stage emission is skewed so each engine's
            # FIFO queue order matches data readiness (no head-of-line
            # blocking), attnV batched per head to limit PE mode switches.
            ot_ps_l = [None] * HPC
            pt_tiles = [[None] * NU for _ in range(HPC)]
            num_sb = [None] * HPC
            den_sb = [None] * HPC
            bc_ps_l = [None] * HPC
            stacked = []
            for p in range(2):
                stk = hbpool.tile([128, T], f32r, tag=f"stk{p}", name=f"stk{p}")
                stacked.append(stk)
            mid = {}

            def span_unit(h, u):
                jc, i0, ni = UNITS[u]
                uk = psUK.tile([128, 4, T], f32, tag="quad", name="uk")
                s_ps = psC.tile([128, 512], f32, tag="psC", name="s_ps")
                for r in range(R):
                    mm(uk[:, r, :ni],
                       kmr_sb[h][32 * r:32 * r + 32, jc * 128:(jc + 1) * 128],
                       uu_sb[h][32 * r:32 * r + 32, i0:i0 + ni],
                       start=True, stop=True,
                       tile_position=(32 * r, 0),
                       skip_group_check=True)
                mm(s_ps[:, :ni],
                   kme4[32 * h:32 * h + 32, jc * 128:(jc + 1) * 128],
                   gt4[32 * h:32 * h + 32, i0:i0 + ni],
                   start=True, stop=True, tile_position=(32 * h, 0))
                sq4 = sqpool.tile([128, 4, 512], f32, tag="sq4", bufs=2)
                nc.scalar.square(sq4[:, :, :ni], uk[:, :, :ni])
                mid[(h, u)] = (sq4, s_ps)

            def tail_unit(h, u, par):
                jc, i0, ni = UNITS[u]
                sq4, s_ps = mid.pop((h, u))
                dd = scr.tile([128, 2, 512], f32, tag="dd", bufs=2)
                eng1 = nc.gpsimd if par == 0 else nc.vector
                eng2 = nc.vector if par == 0 else nc.gpsimd
                eng1.tensor_add(dd[:, :, :ni], sq4[:, 0:2, :ni],
                                sq4[:, 2:4, :ni])
                cc = scr.tile([128, 512], f32, tag="cc", bufs=2)
                eng2.tensor_add(cc[:, :ni], dd[:, 0, :ni], dd[:, 1, :ni])
                ssb = scr.tile([128, 512], f32, tag="ssb", bufs=2)
                nc.vector.scalar_tensor_tensor(
                    ssb[:, :ni], s_ps[:, :ni], 1.0, cc[:, :ni],
                    op0=Alu.bypass, op1=Alu.add)
                pt = ptpool.tile([128, 512], f32r, tag="pt")
                nc.scalar.activation(pt[:, :ni], ssb[:, :ni], Act.Exp,
                                     bias=k2sb[:, h, 2 * jc:2 * jc + 1],
                                     scale=nit[:, 0:1])
                eng1.tensor_mul(pt[:, :128], pt[:, :128], maskd[:])
                pt_tiles[h][u] = pt

            def attn_head(h):
                ot_ps = psD.tile([128, T], f32, tag="psD", name=f"ot{h}")
                ot_ps_l[h] = ot_ps
                for u in range(NU):
                    jc, i0, ni = UNITS[u]
                    mm(ot_ps[:65, i0:i0 + ni],
                       vext[:, jc, h * 65:(h + 1) * 65],
                       pt_tiles[h][u][:, :ni],
                       start=(jc == 0), stop=(u == NU - 1),
                       skip_group_check=True)
                    pt_tiles[h][u] = None

            def drain_head(h):
                ns = obpool.tile([64, T], f32r, tag=f"num{h}", name=f"num{h}")
                nc.vector.tensor_copy(ns[:], ot_ps_l[h][:64, :])
                ds = obpool.tile([1, T], f32r, tag=f"den{h}", name=f"den{h}")
                nc.scalar.copy(ds[:], ot_ps_l[h][64:65, :])
                num_sb[h], den_sb[h] = ns, ds

            def bc_head(h):
                # K=1 broadcast in (32,128) mode: rides the score stream
                bc_ps = psC.tile([128, T], f32, tag="psC", name=f"bc{h}")
                mm(bc_ps[:], ones128[:], den_sb[h][:], start=True, stop=True)
                bc_ps_l[h] = bc_ps

            def norm_head(h):
                p, e = h // 2, h % 2
                rec = scr.tile([64, T], f32, tag="rec", bufs=2)
                nc.vector.reciprocal_approx_fast(out=rec[:],
                                                 in_=bc_ps_l[h][0:64, :])
                nc.vector.tensor_mul(stacked[p][64 * e:64 * e + 64, :],
                                     num_sb[h][:], rec[:])

            ALL = [(h, u) for h in range(HPC) for u in range(NU)]
            for i, (h, u) in enumerate(ALL):
                span_unit(h, u)
                if i >= 1:
                    ph, pu = ALL[i - 1]
                    tail_unit(ph, pu, (i - 1) % 2)
                    if pu == NU - 1:        # head ph fully scored
                        attn_head(ph)
                        drain_head(ph)
                        bc_head(ph)
                        if ph >= 1:
                            norm_head(ph - 1)
            tail_unit(3, NU - 1, 15 % 2)
            attn_head(3)
            drain_head(3)
            bc_head(3)
            norm_head(2)
            norm_head(3)

            # ============ phase 8: (128,128) output projection ============
            for ic in range(TC):
                for ncn in range(2):
                    y_ps = psC.tile([128, 512], f32, tag="psC", name="y_ps")
                    mm(y_ps[:], stacked[0][:, ic * 128:(ic + 1) * 128],
                       wo[0][:, ncn * 512:(ncn + 1) * 512],
                       start=True, stop=False)
                    mm(y_ps[:], stacked[1][:, ic * 128:(ic + 1) * 128],
                       wo[1][:, ncn * 512:(ncn + 1) * 512],
                       start=False, stop=True)
                    y_sb = scr.tile([128, 512], f32, tag="ysb", bufs=2)
                    nc.scalar.copy(y_sb[:], y_ps[:])
                    nc.sync.dma_start(
                        y_d.ap()[ic * 128:(ic + 1) * 128,
                                 ncn * 512:(ncn + 1) * 512],
                        y_sb[:])

    nc.compile()
    return nc


def _r32(a):
    """Round fp32 to fp32r (11-bit mantissa, RNE)."""
    u = np.ascontiguousarray(a, np.float32).view(np.uint32).astype(np.uint64)
    u = (u + 0x7FF + ((u >> 12) & 1)) & 0xFFFFF000
    return u.astype(np.uint32).view(np.float32)


def _rope_tables():
    inv_freq = 1.0 / (10000.0 ** (np.arange(0, DH, 2, dtype=np.float32) / DH))
    t = np.arange(T, dtype=np.float32)
    freqs = t[:, None] * inv_freq[None, :]          # [T, 32]
    return np.cos(freqs), np.sin(freqs)


def _prep_inputs(x, Wq, Wk, Wv, Wo, Wqm, Wkm, Wmetric, temperature):
    x = np.asarray(x, np.float32)
    Wq, Wk, Wv, Wo = (np.asarray(w, np.float32) for w in (Wq, Wk, Wv, Wo))
    Wqm, Wkm = np.asarray(Wqm, np.float32), np.asarray(Wkm, np.float32)
    Wmetric = np.asarray(Wmetric, np.float32)
    temp = float(np.asarray(temperature))

    cosf, sinf = _rope_tables()
    cosr = np.ascontiguousarray(np.tile(cosf.T, (4, 1)))   # [128, T]
    sinr = np.ascontiguousarray(np.tile(sinf.T, (4, 1)))

    bqkA = np.zeros((128, 128), np.float32)
    bqkB = np.zeros((128, 128), np.float32)
    for ee in range(2):
        bqkA[64 * ee:64 * ee + 32, 32 * ee:32 * ee + 32] = Wqm[0:32]
        bqkA[64 * ee + 32:64 * ee + 64, 64 + 32 * ee:96 + 32 * ee] = Wkm[0:32]
        bqkB[64 * ee:64 * ee + 32, 32 * ee:32 * ee + 32] = Wqm[32:64]
        bqkB[64 * ee + 32:64 * ee + 64, 64 + 32 * ee:96 + 32 * ee] = Wkm[32:64]

    wm = np.ascontiguousarray(
        Wmetric.reshape(D, D, R).transpose(0, 2, 1).reshape(D, D * R))
    i4rep = np.ascontiguousarray(np.tile(np.eye(D, dtype=np.float32), (1, 4)))
    wm4 = np.zeros((128, 128), np.float32)
    wm4[0:32] = wm
    wm4[32:64] = wm
    wm4[64:96] = i4rep
    wm4[96:128] = i4rep
    i4rep4 = np.zeros((128, 128), np.float32)
    for s in range(4):
        i4rep4[32 * s:32 * s + 32] = i4rep

    jj, ii = np.meshgrid(np.arange(128), np.arange(128), indexing="ij")
    maskd = (jj <= ii).astype(np.float32)
    it = -1.0 / max(temp, TEMP_MIN)
    nit = np.full((128, 1), it, np.float32)
    negit = np.full((128, 2), it, np.float32)
    gsum = np.zeros((128, 128), np.float32)   # [(r',d'), (r,d)] = [r'==r]
    for a in range(128):
        for bcol in range(128):
            if a // 32 == bcol // 32:
                gsum[a, bcol] = 1.0
    bsum = np.zeros((128, D), np.float32)     # [(r,d), d'] = [d==d']
    for a in range(128):
        bsum[a, a % 32] = 1.0

    in_maps = []
    for c in range(NCORE):
        b, g = c // 4, c % 4
        lh0 = 4 * g
        wqk = np.empty((DM, 512), np.float32)
        for p in range(2):
            for s in range(2):
                m = 2 * p + s
                for ee in range(2):
                    h = lh0 + 2 * p + ee
                    cq = Wq[:, h * 64 + 32 * s: h * 64 + 32 * s + 32]
                    ck = Wk[:, h * 64 + 32 * s: h * 64 + 32 * s + 32]
                    wqk[:, m * 128 + 64 * ee: m * 128 + 64 * ee + 32] = cq
                    wqk[:, m * 128 + 64 * ee + 32: m * 128 + 64 * ee + 64] = ck
        in_maps.append({
            "xt": _r32(x[b].T),
            "wqk": _r32(wqk),
            "wv": _r32(Wv[:, lh0 * 64: lh0 * 64 + 256]),
            "wo": _r32(Wo[lh0 * 64: lh0 * 64 + 256, :]),
            "bqkA": _r32(bqkA),
            "bqkB": _r32(bqkB),
            "wm4": _r32(wm4),
            "i4rep4": i4rep4,
            "cosr": cosr,
            "sinr": sinr,
            "maskd": maskd,
            "nit": nit,
            "negit": negit,
            "gsum": gsum,
            "bsum": bsum,
        })
    return in_maps


def kernel(x, Wq, Wk, Wv, Wo, Wqm, Wkm, Wmetric, temperature, **_):
    from concourse import bass_utils

    if "nc" not in _CACHE:
        _CACHE["nc"] = _build()
    nc = _CACHE["nc"]

    in_maps = _prep_inputs(x, Wq, Wk, Wv, Wo, Wqm, Wkm, Wmetric, temperature)
    res = bass_utils.run_bass_kernel_spmd(nc, in_maps,
                                          core_ids=list(range(NCORE)))
    y = np.zeros((B, T, DM), np.float32)
    for b in range(B):
        acc = res.results[4 * b]["y"].astype(np.float32)
        for g in range(1, 4):
            acc = acc + res.results[4 * b + g]["y"]
        y[b] = acc
    return y


# revision 13
# speedup vs baseline: 1.2642x; 1.0314x over previous
"""DRM attention kernel for 8 Trainium2 NeuronCores.

Sharding: B*H = 32 head-slices; core c handles batch b = c//4 and the 4
heads [4*(c%4), 4*(c%4)+4). Weights replicated (pre-sliced per core on
host). Each core computes its 4 heads' attention output through Wo,
producing a partial [T, DM] for its batch; host sums the 4 partials per
batch.

Score work in transposed layout S^T[j, i] (j = key pos on partitions,
i = query pos on free dim).

Math (per head):
  dist(i,j) = |qm_i-km_j|^2 + sum_r (U_i^T(qm_i-km_j))_r^2
Softmax over j is invariant to any additive term that depends only on
i, so the |qm_i|^2 + |Uq_i|^2 part of the expansion is DROPPED.  What
remains:
  S[j,i] = -2 km_j.(qm_i + w'_i)          (K=32 matmul, w' = U_i Uq_i)
         + |km_j|^2                       (per-j: folded into exp bias)
         + sum_r Uk_ijr^2                 (4 K=32 matmuls, squared)
  p = exp(-(S)/t);  denominator via ones column in V.

Tensor-engine discipline: all K=32 matmuls run in the (32,128) tiling
mode with explicit tile_position so 4 of them occupy the four 32-row
strips of the PE concurrently (uu/kmrep spans, uk spans), and mode
switches (which drain the PE) only happen at a handful of phase
boundaries.
"""

import numpy as np

B, T, DM = 2, 512, 1024
H, DH = 16, 64
D, R = 32, 4
TEMP_MIN = 0.5
NCORE = 8
HPC = 4          # heads per core
TC = 4           # 128-chunks along T (key blocks)

# score units per head: (jc, i0, ni), one per key block jc
UNITS = [(_jc, 128 * _jc, T - 128 * _jc) for _jc in range(TC)]
NU = len(UNITS)   # 4

_CACHE = {}


def _build():
    import concourse.bass as bass
    import concourse.tile as tile
    from concourse import mybir, bacc

    f32 = mybir.dt.float32
    PSUM = bass.MemorySpace.PSUM
    Alu = mybir.AluOpType
    Act = mybir.ActivationFunctionType

    nc = bacc.Bacc("TRN2", target_bir_lowering=False, debug=False)
    f32r = mybir.dt.float32r

    def mm(out, lhsT, rhs, **kw):
        nc.tensor.matmul(out, lhsT.bitcast(f32r), rhs.bitcast(f32r), **kw)

    xt_d = nc.dram_tensor("xt", [DM, T], f32r, kind="ExternalInput")
    wqk_d = nc.dram_tensor("wqk", [DM, 512], f32r, kind="ExternalInput")
    wv_d = nc.dram_tensor("wv", [DM, 256], f32r, kind="ExternalInput")
    wo_d = nc.dram_tensor("wo", [256, DM], f32r, kind="ExternalInput")
    bqkA_d = nc.dram_tensor("bqkA", [128, 128], f32r, kind="ExternalInput")
    bqkB_d = nc.dram_tensor("bqkB", [128, 128], f32r, kind="ExternalInput")
    wm4_d = nc.dram_tensor("wm4", [128, 128], f32r, kind="ExternalInput")
    i4rep4_d = nc.dram_tensor("i4rep4", [128, 128], f32r, kind="ExternalInput")
    cosr_d = nc.dram_tensor("cosr", [128, T], f32, kind="ExternalInput")
    sinr_d = nc.dram_tensor("sinr", [128, T], f32, kind="ExternalInput")
    maskd_d = nc.dram_tensor("maskd", [128, 128], f32, kind="ExternalInput")
    nit_d = nc.dram_tensor("nit", [128, 1], f32, kind="ExternalInput")
    negit_d = nc.dram_tensor("negit", [128, 2], f32r, kind="ExternalInput")
    gsum_d = nc.dram_tensor("gsum", [128, 128], f32r, kind="ExternalInput")
    bsum_d = nc.dram_tensor("bsum", [128, D], f32r, kind="ExternalInput")
    y_d = nc.dram_tensor("y", [T, DM], f32, kind="ExternalOutput")

    with tile.TileContext(nc) as tc:
        with (
            tc.tile_pool(name="const", bufs=1) as cpool,
            tc.tile_pool(name="rope", bufs=4) as rpool,
            tc.tile_pool(name="qkm", bufs=2) as qkmpool,
            tc.tile_pool(name="uu", bufs=4) as uupool,
            tc.tile_pool(name="kmr", bufs=4) as kmrpool,
            tc.tile_pool(name="hb", bufs=1) as hbpool,     # per-head [128,T] persistents
            tc.tile_pool(name="scr", bufs=6) as scr,
            tc.tile_pool(name="sq", bufs=4) as sqpool,
            tc.tile_pool(name="pt", bufs=10) as ptpool,
            tc.tile_pool(name="ob", bufs=1) as obpool,
            tc.tile_pool(name="psUK", bufs=1, space=PSUM) as psUK,
            tc.tile_pool(name="psC", bufs=2, space=PSUM) as psC,
            tc.tile_pool(name="psD", bufs=2, space=PSUM) as psD,
        ):
            # ---- constants / weights ----
            xt = [cpool.tile([128, T], f32r, tag=f"xt{k}", name=f"xt{k}") for k in range(8)]
            wqk = [cpool.tile([128, 512], f32r, tag=f"wqk{k}", name=f"wqk{k}") for k in range(8)]
            wv = [cpool.tile([128, 256], f32r, tag=f"wv{k}", name=f"wv{k}") for k in range(8)]
            wo = [cpool.tile([128, DM], f32r, tag=f"wo{p}", name=f"wo{p}") for p in range(2)]
            bqkA = cpool.tile([128, 128], f32r, tag="bqkA")
            bqkB = cpool.tile([128, 128], f32r, tag="bqkB")
            wm4 = cpool.tile([128, 128], f32r, tag="wm4")
            i4rep4 = cpool.tile([128, 128], f32r, tag="i4rep4")
            cosr = cpool.tile([128, T], f32, tag="cosr")
            sinr = cpool.tile([128, T], f32, tag="sinr")
            maskd = cpool.tile([128, 128], f32, tag="maskd")
            nit = cpool.tile([128, 1], f32, tag="nit")
            negit = cpool.tile([128, 2], f32r, tag="negit")
            gsum = cpool.tile([128, 128], f32r, tag="gsum")
            bsum = cpool.tile([128, D], f32r, tag="bsum")
            ones128 = cpool.tile([1, 128], f32r, tag="ones128")
            warm = cpool.tile([128, 512], f32r, tag="warm")
            vext = cpool.tile([128, TC, 260], f32r, tag="vext")

            xt_r = xt_d.ap().rearrange("(k p) t -> k p t", p=128)
            wqk_r = wqk_d.ap().rearrange("(k p) m -> k p m", p=128)
            wv_r = wv_d.ap().rearrange("(k p) m -> k p m", p=128)
            wo_r = wo_d.ap().rearrange("(k p) m -> k p m", p=128)
            for k in range(8):
                nc.sync.dma_start(xt[k][:], xt_r[k])
                nc.sync.dma_start(wqk[k][:], wqk_r[k])
            nc.sync.dma_start(cosr[:], cosr_d.ap())
            nc.sync.dma_start(sinr[:], sinr_d.ap())
            nc.sync.dma_start(bqkA[:], bqkA_d.ap())
            nc.sync.dma_start(bqkB[:], bqkB_d.ap())
            nc.sync.dma_start(wm4[:], wm4_d.ap())
            nc.sync.dma_start(i4rep4[:], i4rep4_d.ap())
            nc.sync.dma_start(gsum[:], gsum_d.ap())
            nc.sync.dma_start(bsum[:], bsum_d.ap())
            nc.sync.dma_start(maskd[:], maskd_d.ap())
            nc.sync.dma_start(nit[:], nit_d.ap())
            nc.sync.dma_start(negit[:], negit_d.ap())
            for k in range(8):
                nc.sync.dma_start(wv[k][:], wv_r[k])
            for p in range(2):
                nc.sync.dma_start(wo[p][:], wo_r[p])
            nc.gpsimd.memset(ones128[:].bitcast(f32), 1.0)
            nc.gpsimd.memset(warm[:].bitcast(f32), 0.0)
            nc.gpsimd.memset(vext[:].bitcast(f32), 1.0)

            # PE warm-up: ~8us of dummy matmuls while input DMAs stream, so
            # the HAM clock-gate reaches K=8/8 before real work arrives.
            warm_ps = psD.tile([128, 512], f32, tag="psD", name="warm_ps")
            for w in range(20):
                mm(warm_ps[:], warm[:, :128], warm[:],
                   start=(w == 0), stop=(w == 19))

            # ================= phase 1: (128,128) projections =================
            # QK projection + RoPE + qm/km sigmoid, per pair
            qkm_sig = []     # per pair: [128,T] = [qm_e0; qm_e1; km_e0; km_e1]
            for p in range(2):
                ropeAB = []
                for s in range(2):      # dh half: A (first 32) / B (second)
                    m = 2 * p + s
                    qk_ps = psC.tile([128, T], f32, tag="psC", name="qk_ps")
                    for k in range(8):
                        mm(qk_ps[:], wqk[k][:, m * 128:(m + 1) * 128], xt[k][:],
                           start=(k == 0), stop=(k == 7))
                    ropeAB.append(qk_ps)
                A, Bt = ropeAB
                m1 = scr.tile([128, T], f32, tag="scr", bufs=4)
                m2 = scr.tile([128, T], f32, tag="scr", bufs=4)
                nc.vector.tensor_mul(m1[:], A[:], cosr[:])
                nc.vector.tensor_mul(m2[:], Bt[:], sinr[:])
                ropeA = rpool.tile([128, T], f32r, tag="rope")
                nc.vector.tensor_sub(ropeA[:], m1[:], m2[:])
                m3 = scr.tile([128, T], f32, tag="scr", bufs=4)
                m4 = scr.tile([128, T], f32, tag="scr", bufs=4)
                nc.vector.tensor_mul(m3[:], A[:], sinr[:])
                nc.vector.tensor_mul(m4[:], Bt[:], cosr[:])
                ropeB = rpool.tile([128, T], f32r, tag="rope")
                nc.vector.tensor_add(ropeB[:], m3[:], m4[:])

                qkm_ps = psD.tile([128, T], f32, tag="psD", name="qkm_ps")
                mm(qkm_ps[:], bqkA[:], ropeA[:], start=True, stop=False)
                mm(qkm_ps[:], bqkB[:], ropeB[:], start=False, stop=True)
                sig = qkmpool.tile([128, T], f32r, tag="qkm")
                nc.scalar.activation(sig[:], qkm_ps[:], Act.Sigmoid)
                qkm_sig.append(sig)

            # V projection into [v_h | 1] blocks of vext
            for jc in range(TC):
                v_ps = psC.tile([128, 256], f32, tag="psC", name="v_ps")
                for k in range(8):
                    mm(v_ps[:], xt[k][:, jc * 128:(jc + 1) * 128], wv[k][:],
                       start=(k == 0), stop=(k == 7))
                for hl in range(HPC):
                    nc.vector.tensor_copy(vext[:, jc, hl * 65:hl * 65 + 64],
                                          v_ps[:, hl * 64:(hl + 1) * 64])

            # ============ per-pair prep stages ============
            uu_sb = [None] * HPC      # [128, T] f32r, rows (32r+d') = U_r
            kmr_sb = [None] * HPC     # [128, T] f32r, km replicated x4
            tmpc_l = [None] * HPC
            tmp2_l = [None] * HPC
            gt4 = hbpool.tile([128, T], f32r, tag="gt4")
            kme4 = hbpool.tile([128, T], f32r, tag="kme4")
            kmsq4 = hbpool.tile([128, T], f32r, tag="kmsq4")
            k2sb = hbpool.tile([128, 4, 8], f32, tag="k2sb")

            def prep_quad(p):
                # (32,128) mode: uu + kmrep spans, then qm + tmpc
                sig = qkm_sig[p]
                quad = psUK.tile([128, 4, T], f32, tag="quad", name=f"quad{p}")
                for e in range(2):
                    mm(quad[:, e, :], wm4[32 * e:32 * e + 32, :],
                       sig[32 * e:32 * e + 32, :],
                       start=True, stop=True, tile_position=(32 * e, 0),
                       skip_group_check=True)
                for e in range(2):
                    mm(quad[:, 2 + e, :], i4rep4[64 + 32 * e:96 + 32 * e, :],
                       sig[64 + 32 * e:96 + 32 * e, :],
                       start=True, stop=True, tile_position=(64 + 32 * e, 0),
                       skip_group_check=True)
                for e in range(2):
                    h = 2 * p + e
                    u = uupool.tile([128, T], f32r, tag="uu", name=f"uu{h}")
                    nc.vector.tensor_copy(u[:], quad[:, e, :])
                    uu_sb[h] = u
                    kr = kmrpool.tile([128, T], f32r, tag="kmr", name=f"kmr{h}")
                    nc.vector.tensor_copy(kr[:], quad[:, 2 + e, :])
                    kmr_sb[h] = kr

            def prep_qm(p):
                sig = qkm_sig[p]
                for e in range(2):
                    h = 2 * p + e
                    pool = psC if e == 0 else psD
                    qm_ps = pool.tile([128, T], f32, tag=pool.name,
                                      name=f"qm_ps{h}")
                    mm(qm_ps[:], i4rep4[32 * e:32 * e + 32, :],
                       sig[32 * e:32 * e + 32, :],
                       start=True, stop=True, tile_position=(32 * e, 0))
                    t = scr.tile([128, T], f32r, tag="tmpc", bufs=4)
                    nc.vector.scalar_tensor_tensor(
                        t[:], qm_ps[:], 1.0, uu_sb[h][:],
                        op0=Alu.bypass, op1=Alu.mult)
                    tmpc_l[h] = t
                    nc.gpsimd.tensor_mul(
                        kmsq4[32 * h:32 * h + 32, :],
                        sig[64 + 32 * e:96 + 32 * e, :],
                        sig[64 + 32 * e:96 + 32 * e, :])
                    nc.vector.tensor_scalar_mul(
                        kme4[32 * h:32 * h + 32, :],
                        sig[64 + 32 * e:96 + 32 * e, :], -2.0)

            def prep_gsum(p):
                # (128,128) mode
                for e in range(2):
                    h = 2 * p + e
                    pool = psC if e == 0 else psD
                    uq_ps = pool.tile([128, T], f32, tag=pool.name,
                                      name=f"uq_ps{h}")
                    mm(uq_ps[:], gsum[:], tmpc_l[h][:], start=True, stop=True)
                    t2 = scr.tile([128, T], f32r, tag="tmp2", bufs=4)
                    nc.vector.scalar_tensor_tensor(
                        t2[:], uq_ps[:], 1.0, uu_sb[h][:],
                        op0=Alu.bypass, op1=Alu.mult)
                    tmp2_l[h] = t2

            def prep_wpt(p):
                # (128,32) mode
                sig = qkm_sig[p]
                for e in range(2):
                    h = 2 * p + e
                    pool = psC if e == 0 else psD
                    wpt_ps = pool.tile([32, T], f32, tag=pool.name,
                                       name=f"wpt_ps{h}")
                    mm(wpt_ps[:], bsum[:], tmp2_l[h][:], start=True, stop=True)
                    nc.vector.scalar_tensor_tensor(
                        gt4[32 * h:32 * h + 32, :], wpt_ps[:],
                        1.0, sig[32 * e:32 * e + 32, :],
                        op0=Alu.bypass, op1=Alu.add)

            def prep_k2(p):
                # (32,128) mode
                k2c = psC.tile([128, 512], f32, tag="psC", name=f"k2c{p}")
                k2d = psD.tile([128, 512], f32, tag="psD", name=f"k2d{p}")
                for e in range(2):
                    h = 2 * p + e
                    dst = k2c if e == 0 else k2d
                    for jc in range(TC):
                        mm(dst[:, 2 * jc:2 * jc + 2],
                           kmsq4[32 * h:32 * h + 32, jc * 128:(jc + 1) * 128],
                           negit[32 * h:32 * h + 32, :],
                           start=(jc == 0), stop=True,
                           tile_position=(32 * h, 0),
                           skip_group_check=True)
                nc.vector.tensor_copy(k2sb[:, 2 * p, :], k2c[:, 0:8])
                nc.vector.tensor_copy(k2sb[:, 2 * p + 1, :], k2d[:, 0:8])

            # pair 0 prep runs up front; pair 1 prep overlaps the score
            # stream of heads 0/1 (32-mode pieces ride between score spans,
            # 128-mode pieces sit at the attnV bursts).
            prep_quad(0)
            prep_qm(0)
            prep_gsum(0)
            prep_wpt(0)
            prep_k2(0)

            # ============ phase 6: score loop ============
            # software-pipelined: stage emission is skewed so each engine's
            # FIFO queue order matches data readiness (no head-of-line
            # blocking), attnV batched per head to limit PE mode switches.
            ot_ps_l = [None] * HPC
            pt_tiles = [[None] * NU for _ in range(HPC)]
            num_sb = [None] * HPC
            den_sb = [None] * HPC
            bc_ps_l = [None] * HPC
            stacked = []
            for p in range(2):
                stk = hbpool.tile([128, T], f32r, tag=f"stk{p}", name=f"stk{p}")
                stacked.append(stk)
            mid = {}

            def span_unit(h, u):
                jc, i0, ni = UNITS[u]
                uk = psUK.tile([128, 4, T], f32, tag="quad", name="uk")
                s_ps = psC.tile([128, 512], f32, tag="psC", name="s_ps")
                for r in range(R):
                    mm(uk[:, r, :ni],
                       kmr_sb[h][32 * r:32 * r + 32, jc * 128:(jc + 1) * 128],
                       uu_sb[h][32 * r:32 * r + 32, i0:i0 + ni],
                       start=True, stop=True,
                       tile_position=(32 * r, 0),
                       skip_group_check=True)
                mm(s_ps[:, :ni],
                   kme4[32 * h:32 * h + 32, jc * 128:(jc + 1) * 128],
                   gt4[32 * h:32 * h + 32, i0:i0 + ni],
                   start=True, stop=True, tile_position=(32 * h, 0))
                sq4 = sqpool.tile([128, 4, 512], f32, tag="sq4", bufs=2)
                nc.scalar.square(sq4[:, :, :ni], uk[:, :, :ni])
                mid[(h, u)] = (sq4, s_ps)

            def tail_unit(h, u, par):
                jc, i0, ni = UNITS[u]
                sq4, s_ps = mid.pop((h, u))
                dd = scr.tile([128, 2, 512], f32, tag="dd", bufs=2)
                eng1 = nc.gpsimd if par == 0 else nc.vector
                eng2 = nc.vector if par == 0 else nc.gpsimd
                eng1.tensor_add(dd[:, :, :ni], sq4[:, 0:2, :ni],
                                sq4[:, 2:4, :ni])
                cc = scr.tile([128, 512], f32, tag="cc", bufs=2)
                eng2.tensor_add(cc[:, :ni], dd[:, 0, :ni], dd[:, 1, :ni])
                ssb = scr.tile([128, 512], f32, tag="ssb", bufs=2)
                nc.vector.scalar_tensor_tensor(
                    ssb[:, :ni], s_ps[:, :ni], 1.0, cc[:, :ni],
                    op0=Alu.bypass, op1=Alu.add)
                pt = ptpool.tile([128, 512], f32r, tag="pt")
                nc.scalar.activation(pt[:, :ni], ssb[:, :ni], Act.Exp,
                                     bias=k2sb[:, h, 2 * jc:2 * jc + 1],
                                     scale=nit[:, 0:1])
                eng1.tensor_mul(pt[:, :128], pt[:, :128], maskd[:])
                pt_tiles[h][u] = pt

            def attn_head(h):
                ot_ps = psD.tile([128, T], f32, tag="psD", name=f"ot{h}")
                ot_ps_l[h] = ot_ps
                for u in range(NU):
                    jc, i0, ni = UNITS[u]
                    mm(ot_ps[:65, i0:i0 + ni],
                       vext[:, jc, h * 65:(h + 1) * 65],
                       pt_tiles[h][u][:, :ni],
                       start=(jc == 0), stop=(u == NU - 1),
                       skip_group_check=True)
                    pt_tiles[h][u] = None

            def drain_head(h):
                ns = obpool.tile([64, T], f32r, tag=f"num{h}", name=f"num{h}")
                nc.vector.tensor_copy(ns[:], ot_ps_l[h][:64, :])
                ds = obpool.tile([1, T], f32r, tag=f"den{h}", name=f"den{h}")
                nc.scalar.copy(ds[:], ot_ps_l[h][64:65, :])
                num_sb[h], den_sb[h] = ns, ds

            def bc_head(h):
                # K=1 broadcast in (32,128) mode: rides the score stream
                bc_ps = psC.tile([128, T], f32, tag="psC", name=f"bc{h}")
                mm(bc_ps[:], ones128[:], den_sb[h][:], start=True, stop=True)
                bc_ps_l[h] = bc_ps

            def norm_head(h):
                p, e = h // 2, h % 2
                rec = scr.tile([64, T], f32, tag="rec", bufs=2)
                nc.vector.reciprocal_approx_fast(out=rec[:],
                                                 in_=bc_ps_l[h][0:64, :])
                nc.vector.tensor_mul(stacked[p][64 * e:64 * e + 64, :],
                                     num_sb[h][:], rec[:])

            ALL = [(h, u) for h in range(HPC) for u in range(NU)]
            for i, (h, u) in enumerate(ALL):
                span_unit(h, u)
                if (h, u) == (0, 1):
                    prep_quad(1)       # 32-mode: rides the score stream
                if (h, u) == (0, 3):
                    prep_qm(1)         # 32-mode
                if (h, u) == (1, 2):
                    prep_k2(1)         # 32-mode
                if i >= 1:
                    ph, pu = ALL[i - 1]
                    tail_unit(ph, pu, (i - 1) % 2)
                    if pu == NU - 1:        # head ph fully scored
                        attn_head(ph)
                        if ph == 0:
                            prep_gsum(1)   # 128-mode, at the attnV burst
                            prep_wpt(1)
                        drain_head(ph)
                        bc_head(ph)
                        if ph >= 1:
                            norm_head(ph - 1)
            tail_unit(3, NU - 1, 15 % 2)
            attn_head(3)
            drain_head(3)
            bc_head(3)
            norm_head(2)
            norm_head(3)

            # ============ phase 8: (128,128) output projection ============
            for ic in range(TC):
                for ncn in range(2):
                    y_ps = psC.tile([128, 512], f32, tag="psC", name="y_ps")
                    mm(y_ps[:], stacked[0][:, ic * 128:(ic + 1) * 128],
                       wo[0][:, ncn * 512:(ncn + 1) * 512],
                       start=True, stop=False)
                    mm(y_ps[:], stacked[1][:, ic * 128:(ic + 1) * 128],
                       wo[1][:, ncn * 512:(ncn + 1) * 512],
                       start=False, stop=True)
                    y_sb = scr.tile([128, 512], f32, tag="ysb", bufs=2)
                    nc.scalar.copy(y_sb[:], y_ps[:])
                    nc.sync.dma_start(
                        y_d.ap()[ic * 128:(ic + 1) * 128,
                                 ncn * 512:(ncn + 1) * 512],
                        y_sb[:])

    nc.compile()
    return nc


def _r32(a):
    """Round fp32 to fp32r (11-bit mantissa, RNE)."""
    u = np.ascontiguousarray(a, np.float32).view(np.uint32).astype(np.uint64)
    u = (u + 0x7FF + ((u >> 12) & 1)) & 0xFFFFF000
    return u.astype(np.uint32).view(np.float32)


def _rope_tables():
    inv_freq = 1.0 / (10000.0 ** (np.arange(0, DH, 2, dtype=np.float32) / DH))
    t = np.arange(T, dtype=np.float32)
    freqs = t[:, None] * inv_freq[None, :]          # [T, 32]
    return np.cos(freqs), np.sin(freqs)


def _prep_inputs(x, Wq, Wk, Wv, Wo, Wqm, Wkm, Wmetric, temperature):
    x = np.asarray(x, np.float32)
    Wq, Wk, Wv, Wo = (np.asarray(w, np.float32) for w in (Wq, Wk, Wv, Wo))
    Wqm, Wkm = np.asarray(Wqm, np.float32), np.asarray(Wkm, np.float32)
    Wmetric = np.asarray(Wmetric, np.float32)
    temp = float(np.asarray(temperature))

    cosf, sinf = _rope_tables()
    cosr = np.ascontiguousarray(np.tile(cosf.T, (4, 1)))   # [128, T]
    sinr = np.ascontiguousarray(np.tile(sinf.T, (4, 1)))

    bqkA = np.zeros((128, 128), np.float32)
    bqkB = np.zeros((128, 128), np.float32)
    for ee in range(2):
        bqkA[64 * ee:64 * ee + 32, 32 * ee:32 * ee + 32] = Wqm[0:32]
        bqkA[64 * ee + 32:64 * ee + 64, 64 + 32 * ee:96 + 32 * ee] = Wkm[0:32]
        bqkB[64 * ee:64 * ee + 32, 32 * ee:32 * ee + 32] = Wqm[32:64]
        bqkB[64 * ee + 32:64 * ee + 64, 64 + 32 * ee:96 + 32 * ee] = Wkm[32:64]

    wm = np.ascontiguousarray(
        Wmetric.reshape(D, D, R).transpose(0, 2, 1).reshape(D, D * R))
    i4rep = np.ascontiguousarray(np.tile(np.eye(D, dtype=np.float32), (1, 4)))
    wm4 = np.zeros((128, 128), np.float32)
    wm4[0:32] = wm
    wm4[32:64] = wm
    wm4[64:96] = i4rep
    wm4[96:128] = i4rep
    i4rep4 = np.zeros((128, 128), np.float32)
    for s in range(4):
        i4rep4[32 * s:32 * s + 32] = i4rep

    jj, ii = np.meshgrid(np.arange(128), np.arange(128), indexing="ij")
    maskd = (jj <= ii).astype(np.float32)
    it = -1.0 / max(temp, TEMP_MIN)
    nit = np.full((128, 1), it, np.float32)
    negit = np.full((128, 2), it, np.float32)
    gsum = np.zeros((128, 128), np.float32)   # [(r',d'), (r,d)] = [r'==r]
    for a in range(128):
        for bcol in range(128):
            if a // 32 == bcol // 32:
                gsum[a, bcol] = 1.0
    bsum = np.zeros((128, D), np.float32)     # [(r,d), d'] = [d==d']
    for a in range(128):
        bsum[a, a % 32] = 1.0

    in_maps = []
    for c in range(NCORE):
        b, g = c // 4, c % 4
        lh0 = 4 * g
        wqk = np.empty((DM, 512), np.float32)
        for p in range(2):
            for s in range(2):
                m = 2 * p + s
                for ee in range(2):
                    h = lh0 + 2 * p + ee
                    cq = Wq[:, h * 64 + 32 * s: h * 64 + 32 * s + 32]
                    ck = Wk[:, h * 64 + 32 * s: h * 64 + 32 * s + 32]
                    wqk[:, m * 128 + 64 * ee: m * 128 + 64 * ee + 32] = cq
                    wqk[:, m * 128 + 64 * ee + 32: m * 128 + 64 * ee + 64] = ck
        in_maps.append({
            "xt": _r32(x[b].T),
            "wqk": _r32(wqk),
            "wv": _r32(Wv[:, lh0 * 64: lh0 * 64 + 256]),
            "wo": _r32(Wo[lh0 * 64: lh0 * 64 + 256, :]),
            "bqkA": _r32(bqkA),
            "bqkB": _r32(bqkB),
            "wm4": _r32(wm4),
            "i4rep4": i4rep4,
            "cosr": cosr,
            "sinr": sinr,
            "maskd": maskd,
            "nit": nit,
            "negit": negit,
            "gsum": gsum,
            "bsum": bsum,
        })
    return in_maps


def kernel(x, Wq, Wk, Wv, Wo, Wqm, Wkm, Wmetric, temperature, **_):
    from concourse import bass_utils

    if "nc" not in _CACHE:
        _CACHE["nc"] = _build()
    nc = _CACHE["nc"]

    in_maps = _prep_inputs(x, Wq, Wk, Wv, Wo, Wqm, Wkm, Wmetric, temperature)
    res = bass_utils.run_bass_kernel_spmd(nc, in_maps,
                                          core_ids=list(range(NCORE)))
    y = np.zeros((B, T, DM), np.float32)
    for b in range(B):
        acc = res.results[4 * b]["y"].astype(np.float32)
        for g in range(1, 4):
            acc = acc + res.results[4 * b + g]["y"]
        y[b] = acc
    return y
